# revision 92
# baseline (speedup 1.0000x reference)
"""MBConv (expand 1x1 + BN/ReLU, depthwise 3x3 + BN/ReLU, project 1x1 + BN,
residual) on 8 Trainium2 NeuronCores, data-parallel over the batch.

Strategy
--------
- BN folded into conv weights/biases on the host.
- conv1/conv3 are bf16 matmuls on the tensor engine (fp32 matmul is 4x
  slower; the residual keeps exact fp32 x, so bf16 costs ~5e-4 rel err).
- depthwise 3x3 = 9 shifted taps over a zero-padded [58,58] layout, split
  across engines by a per-(image, ctile) job table:
    * "pe":  9 diagonal-weight matmuls accumulating in PSUM,
    * "dve": fused scalar_tensor_tensor chain, GpSimd helping on the
             4B-misaligned middle-column taps,
    * "act": ACT makes scaled tap products, DVE sums them at bf16 2x.
- Evictions (PSUM -> SBUF) fuse the BN bias + ReLU via ACT/DVE.
- conv3 eviction fuses bias + residual add in one scalar_tensor_tensor.
"""

import sys

for _p in ("/opt/trn_rl_repo", "/root/.axon_site/_ro/trn_rl_repo"):
    if _p not in sys.path:
        sys.path.append(_p)

import ml_dtypes
import numpy as np

import concourse.bass as bass
import concourse.mybir as mybir
import concourse.tile as tile
from concourse import bacc
from concourse.bass_utils import run_bass_kernel_spmd

EPS = 1e-5
N_CORES = 8
NIMG = 4            # images per core
C = 96              # in/out channels
M = 576             # expanded channels
H = W = 56
PIX = H * W         # 3136
WP = 58             # padded width/height
PPIX = WP * WP      # 3364
RCH = 8             # rows per chunk
CHUNK = RCH * W     # 448
NCH = H // RCH      # 7 chunks
CTILES = [(0, 128), (128, 128), (256, 128), (384, 128), (512, 64)]
NCT = len(CTILES)
KT3 = [(0, 128), (128, 128), (256, 128), (384, 128), (512, 64)]  # conv3 k-tiles

F32 = mybir.dt.float32
F32R = mybir.dt.float32r
BF16 = mybir.dt.bfloat16
AOP = mybir.AluOpType
AF = mybir.ActivationFunctionType

# tap engine assignment per job (img n, ctile m):
#   "pe"  — 9 diagonal matmuls accumulating in PSUM
#   "dve" — scalar_tensor_tensor chain on DVE (1x rate)
#   "act" — ACT makes scaled tap products, DVE sums them at bf16 2x
JOB = [
    ["pe", "act", "dve", "pe", "dve"],
    ["pe", "pe", "act", "dve", "pe"],
    ["act", "pe", "pe", "dve", "pe"],
    ["pe", "pe", "act", "pe", "dve"],
]

_CACHE = {}


def _r(ap):
    """bitcast an fp32 AP to float32r for full-rate matmul"""
    return ap.bitcast(F32R)


def _build(stages=("conv1", "dwpe", "dwdve", "conv3"), nimg=NIMG):
    stages = set(stages)
    nc = bacc.Bacc("TRN2", target_bir_lowering=False, debug=False)

    # ---- dram tensors -------------------------------------------------
    x_d = nc.dram_tensor("x", [nimg, C, PIX], F32, kind="ExternalInput").ap()
    x16_d = nc.dram_tensor("x16", [nimg, C, PIX], BF16, kind="ExternalInput").ap()
    w1t_d = nc.dram_tensor("w1t", [C, M], BF16, kind="ExternalInput").ap()
    dg_d = nc.dram_tensor("dg", [128, NCT * 9 * 128], BF16, kind="ExternalInput").ap()
    wds_d = nc.dram_tensor("wds", [128, NCT * 9], F32, kind="ExternalInput").ap()
    w3t_d = nc.dram_tensor("w3t", [128, NCT * 128], BF16, kind="ExternalInput").ap()
    b1_d = nc.dram_tensor("b1c", [128, NCT], F32, kind="ExternalInput").ap()
    b2_d = nc.dram_tensor("b2c", [128, NCT], F32, kind="ExternalInput").ap()
    b3_d = nc.dram_tensor("b3c", [C, 1], F32, kind="ExternalInput").ap()
    y_d = nc.dram_tensor("y", [nimg, C, PIX], F32, kind="ExternalOutput").ap()

    with tile.TileContext(nc) as tc:
        with (
            tc.tile_pool(name="const", bufs=1) as constp,
            tc.tile_pool(name="o1", bufs=1) as o1p,
            tc.tile_pool(name="o2", bufs=1) as o2p,
            tc.tile_pool(name="xin", bufs=2) as xp,
            tc.tile_pool(name="xc", bufs=2) as xcp,
            tc.tile_pool(name="gac", bufs=1) as gaccp,
            tc.tile_pool(name="acc", bufs=1) as accp,
            tc.tile_pool(name="tmp", bufs=2) as tmpp,
            tc.tile_pool(name="o3", bufs=2) as o3p,
            tc.tile_pool(name="ps1", bufs=2, space="PSUM") as ps1p,
            tc.tile_pool(name="psd", bufs=2, space="PSUM") as psdp,
            tc.tile_pool(name="ps3", bufs=2, space="PSUM") as ps3p,
        ):
            # ---- constants / weights -------------------------------------
            # w1t/b1c + image-0 input first so conv1(0) isn't queued behind
            # the bulky dg/w3t constant DMAs
            w1t = constp.tile([C, M], BF16)
            nc.sync.dma_start(w1t[:], w1t_d[:])
            b1c = constp.tile([128, NCT], F32)
            nc.sync.dma_start(b1c[:], b1_d[:])
            x_bf_pre = xp.tile([C, PIX], BF16, name="x_bf_pre", tag="x_bf")
            for q in range(2):
                nc.sync.dma_start(
                    x_bf_pre[q * 48 : (q + 1) * 48, :],
                    x16_d[0, q * 48 : (q + 1) * 48, :],
                )
            # warm up the PE clock during the initial DMAs (dummy matmuls,
            # gated only on the small w1t load)
            warm = ps1p.tile([C, 1024], F32, name="warm", tag="ps1")
            for _ in range(14):
                nc.tensor.matmul(
                    warm[:, 0:448], w1t[:, 0:C], w1t[:, 0:448],
                    start=True, stop=True,
                )
            dg = constp.tile([128, NCT * 9 * 128], BF16)
            for m in range(NCT):
                nc.sync.dma_start(
                    dg[:, m * 9 * 128 : (m + 1) * 9 * 128],
                    dg_d[:, m * 9 * 128 : (m + 1) * 9 * 128],
                )
            wds = constp.tile([128, NCT * 9], F32)
            nc.sync.dma_start(wds[:], wds_d[:])
            w3t = constp.tile([128, NCT * 128], BF16)
            nc.sync.dma_start(w3t[:], w3t_d[:])
            b2c = constp.tile([128, NCT], F32)
            nc.sync.dma_start(b2c[:], b2_d[:])
            b3c = constp.tile([C, 1], F32)
            nc.sync.dma_start(b3c[:], b3_d[:])

            # ---- persistent activation buffers ---------------------------
            # padded conv1 outputs, double-buffered per ctile (slot = n % 2);
            # zero borders via one-time memset
            o1 = []
            o2 = []
            for m, (cs, P) in enumerate(CTILES):
                pair = []
                pair2 = []
                for s in range(2):
                    t1 = o1p.tile([128, PPIX], BF16, name=f"o1_{m}_{s}")
                    t1r = t1.rearrange("p (r c) -> p r c", c=WP)
                    # zero only the border cells (top/bottom rows, l/r columns)
                    nc.gpsimd.memset(t1[:, 0:WP], 0.0)
                    nc.gpsimd.memset(t1[:, PPIX - WP : PPIX], 0.0)
                    nc.gpsimd.memset(t1r[:, :, 0:1], 0.0)
                    nc.gpsimd.memset(t1r[:, :, WP - 1 : WP], 0.0)
                    pair.append(t1)
                    t2 = o2p.tile([128, PIX], BF16, name=f"o2_{m}_{s}")
                    pair2.append(t2)
                o1.append(pair)
                o2.append(pair2)

            # ---- per-image pipeline --------------------------------------
            xbf_next = {0: x_bf_pre}
            for n in range(nimg):
                x_bf = xbf_next.pop(n)
                if n + 1 < nimg:
                    # prefetch next image before this image's output DMAs
                    # enter the sync queue (in-order sequencer would stall it)
                    nx = xp.tile([C, PIX], BF16, name="x_bf", tag="x_bf")
                    for q in range(2):
                        nc.sync.dma_start(
                            nx[q * 48 : (q + 1) * 48, :],
                            x16_d[n + 1, q * 48 : (q + 1) * 48, :],
                        )
                    xbf_next[n + 1] = nx

                # conv1: out1 = relu(W1' @ x + b1'), written into padded layout.
                # psum tiles span 2 banks; two row-chunks per eviction.
                for m, (cs, P) in enumerate(CTILES):
                    if "conv1" not in stages:
                        break
                    o1r = o1[m][n % 2].rearrange("p (r c) -> p r c", c=WP)
                    for j0 in range(0, NCH, 2):
                        npair = min(2, NCH - j0)
                        ps = ps1p.tile([P, 1024], F32, name="ps1", tag="ps1")
                        for g in range(npair):
                            nc.tensor.matmul(
                                ps[:, g * 512 : g * 512 + CHUNK],
                                w1t[:, cs : cs + P],
                                x_bf[:, (j0 + g) * CHUNK : (j0 + g + 1) * CHUNK],
                                start=True,
                                stop=True,
                            )
                        if npair == 2:
                            src = (
                                ps.rearrange("p (g x) -> p g x", g=2)[:, :, 0:CHUNK]
                                .rearrange("p g (r c) -> p g r c", c=56)
                            )
                            dst = o1r[
                                0:P, j0 * RCH + 1 : j0 * RCH + 2 * RCH + 1, 1:57
                            ].rearrange("p (g r) c -> p g r c", g=2)
                        else:
                            src = ps[:, 0:CHUNK]
                            dst = o1r[0:P, j0 * RCH + 1 : j0 * RCH + RCH + 1, 1:57]
                        nc.scalar.activation(
                            dst, src, AF.Relu, bias=b1c[0:P, m : m + 1]
                        )

                # depthwise 3x3 + BN + ReLU
                for m, (cs, P) in enumerate(CTILES):
                    o1r = o1[m][n % 2].rearrange("p (r c) -> p r c", c=WP)
                    jb = JOB[n][m]
                    o2t = o2[m][n % 2]
                    if ("dwpe" if jb == "pe" else "dwdve") not in stages:
                        nc.scalar.activation(
                            o2t[0:P, :], o1r[0:P, 0:56, 0:56], AF.Relu
                        )
                        continue

                    def tap(k):
                        ky, kx = divmod(k, 3)
                        return o1r[0:P, ky : ky + 56, kx : kx + 56]

                    wd1 = lambda k: wds[0:P, m * 9 + k : m * 9 + k + 1]

                    if jb == "pe":
                        for j in range(NCH):
                            ps = psdp.tile([P, CHUNK], F32, name="psd", tag="psd")
                            for k in range(9):
                                ky, kx = divmod(k, 3)
                                nc.tensor.matmul(
                                    ps[:],
                                    dg[0:P, (m * 9 + k) * 128 : (m * 9 + k) * 128 + P],
                                    o1r[0:P, j * RCH + ky : j * RCH + ky + 8, kx : kx + 56],
                                    start=(k == 0),
                                    stop=(k == 8),
                                )
                            dst2 = o2t[0:P, j * CHUNK : (j + 1) * CHUNK]
                            if (n + m + j) % 2 == 0:
                                nc.scalar.activation(
                                    dst2, ps[:], AF.Relu, bias=b2c[0:P, m : m + 1]
                                )
                            else:
                                nc.vector.tensor_scalar(
                                    dst2, ps[:], b2c[0:P, m : m + 1], 0.0,
                                    AOP.add, AOP.max,
                                )
                    elif jb == "act":
                        # ACT computes per-tap scaled products; DVE sums at 2x
                        acc = accp.tile([128, PIX], BF16, name="acc", tag="acca", bufs=2)
                        nc.scalar.activation(
                            acc[0:P, :],
                            tap(0),
                            AF.Identity,
                            bias=b2c[0:P, m : m + 1],
                            scale=wd1(0),
                        )
                        for k in range(1, 9):
                            t = tmpp.tile([128, PIX], BF16, name="tp", tag="tmp")
                            nc.scalar.activation(
                                t[0:P, :], tap(k), AF.Copy, scale=wd1(k)
                            )
                            nc.vector.tensor_add(acc[0:P, :], acc[0:P, :], t[0:P, :])
                        nc.vector.tensor_scalar_max(o2t[0:P, :], acc[0:P, :], 0.0)
                    else:
                        # GpSimd takes the three 4B-misaligned kx==1 taps
                        # (mul + add pairs; no fused STT on Pool); DVE takes
                        # the six aligned taps + merge.
                        gacc = gaccp.tile([128, PIX], BF16, name="gacc", tag="gacc")
                        gtm = gaccp.tile([128, PIX], BF16, name="gtm", tag="gtm", bufs=1)
                        nc.gpsimd.tensor_scalar_mul(gacc[0:P, :], tap(1), wd1(1))
                        for k in (4, 7):
                            nc.gpsimd.tensor_scalar_mul(gtm[0:P, :], tap(k), wd1(k))
                            nc.gpsimd.tensor_add(
                                gacc[0:P, :], gacc[0:P, :], gtm[0:P, :]
                            )
                        acc = accp.tile([128, PIX], BF16, name="acc", tag="acc")
                        nc.vector.tensor_scalar(
                            acc[0:P, :], tap(0), wd1(0), b2c[0:P, m : m + 1],
                            AOP.mult, AOP.add,
                        )
                        for k in (2, 3, 5, 6, 8):
                            nc.vector.scalar_tensor_tensor(
                                acc[0:P, :], tap(k), wd1(k), acc[0:P, :],
                                AOP.mult, AOP.add,
                            )
                        nc.vector.tensor_add(acc[0:P, :], acc[0:P, :], gacc[0:P, :])
                        nc.vector.tensor_scalar_max(o2t[0:P, :], acc[0:P, :], 0.0)

                # conv3 + bias + residual; x residual fetched per chunk
                if "conv3" not in stages:
                    o3f = o3p.tile([C, PIX], BF16, name="o3f", tag="o3f")
                    nc.vector.tensor_copy(o3f[:], x_bf[:])
                    nc.sync.dma_start(y_d[n].bitcast(BF16)[:, 0:PIX], o3f[:])
                xcs = {}

                def fetch_xc(j):
                    if j < NCH:
                        t = xcp.tile([C, CHUNK], F32, name="xc", tag="xc")
                        nc.sync.dma_start(
                            t[:], x_d[n, :, j * CHUNK : (j + 1) * CHUNK]
                        )
                        xcs[j] = t

                if "conv3" in stages:
                    fetch_xc(0)
                    fetch_xc(1)
                for j in range(NCH):
                    if "conv3" not in stages:
                        break
                    xc = xcs.pop(j)
                    ps = ps3p.tile([128, CHUNK], F32, name="ps3", tag="ps3")
                    for kt, (ks, K) in enumerate(KT3):
                        nc.tensor.matmul(
                            ps[:],
                            w3t[0:K, kt * 128 : kt * 128 + 128],
                            o2[kt][n % 2][0:K, j * CHUNK : (j + 1) * CHUNK],
                            start=(kt == 0),
                            stop=(kt == len(KT3) - 1),
                        )
                    o3 = o3p.tile([C, CHUNK], F32, name="o3", tag="o3")
                    nc.vector.scalar_tensor_tensor(
                        o3[:],
                        ps[0:C, :],
                        b3c[:],
                        xc[:],
                        AOP.add,
                        AOP.add,
                    )
                    fetch_xc(j + 2)  # issue before y so the in-order DMA
                    # sequencer doesn't stall the next residual fetch
                    nc.sync.dma_start(
                        y_d[n, :, j * CHUNK : (j + 1) * CHUNK], o3[:]
                    )

    nc.compile()
    return nc


def _fold_bn(inputs):
    """fold BN params into conv weights/biases; build device-side arrays"""
    f = lambda k: np.asarray(inputs[k], np.float32)
    w1, g1, b1, m1, v1 = f("w1"), f("g1"), f("b1"), f("m1"), f("v1")
    wd, g2, b2, m2, v2 = f("wd"), f("g2"), f("b2"), f("m2"), f("v2")
    w3, g3, b3, m3, v3 = f("w3"), f("g3"), f("b3"), f("m3"), f("v3")

    s1 = g1 / np.sqrt(v1 + EPS)
    W1p = w1[:, :, 0, 0] * s1[:, None]              # [M, C]
    b1p = b1 - m1 * s1                              # [M]
    s2 = g2 / np.sqrt(v2 + EPS)
    wdp = wd[:, 0] * s2[:, None, None]              # [M, 3, 3]
    b2p = b2 - m2 * s2
    s3 = g3 / np.sqrt(v3 + EPS)
    W3p = w3[:, :, 0, 0] * s3[:, None]              # [C, M]
    b3p = b3 - m3 * s3

    w1t = np.ascontiguousarray(W1p.T).astype(ml_dtypes.bfloat16)  # [C, M] lhsT

    dgm = np.zeros((128, NCT * 9 * 128), np.float32)
    wds = np.zeros((128, NCT * 9), np.float32)
    for m, (cs, P) in enumerate(CTILES):
        for k in range(9):
            ky, kx = divmod(k, 3)
            blk = dgm[:P, (m * 9 + k) * 128 : (m * 9 + k) * 128 + P]
            np.fill_diagonal(blk, wdp[cs : cs + P, ky, kx])
            wds[:P, m * 9 + k] = wdp[cs : cs + P, ky, kx]
    dgm = dgm.astype(ml_dtypes.bfloat16)

    w3t = np.zeros((128, NCT * 128), np.float32)
    for kt, (ks, K) in enumerate(KT3):
        w3t[:K, kt * 128 : kt * 128 + C] = W3p.T[ks : ks + K, :]
    w3t = w3t.astype(ml_dtypes.bfloat16)

    b1c = np.zeros((128, NCT), np.float32)
    b2c = np.zeros((128, NCT), np.float32)
    for m, (cs, P) in enumerate(CTILES):
        b1c[:P, m] = b1p[cs : cs + P]
        b2c[:P, m] = b2p[cs : cs + P]
    b3c = b3p.reshape(C, 1).astype(np.float32)

    return dict(w1t=w1t, dg=dgm, wds=wds, w3t=w3t, b1c=b1c, b2c=b2c, b3c=b3c)


def kernel(**inputs):
    if "nc" not in _CACHE:
        _CACHE["nc"] = _build()
    nc = _CACHE["nc"]

    params = _fold_bn(inputs)
    x = np.asarray(inputs["x"], np.float32)
    B = x.shape[0]
    xr = x.reshape(N_CORES, NIMG, C, PIX)

    in_maps = [
        dict(
            x=np.ascontiguousarray(xr[c]),
            x16=np.ascontiguousarray(xr[c]).astype(ml_dtypes.bfloat16),
            **params,
        )
        for c in range(N_CORES)
    ]
    res = run_bass_kernel_spmd(nc, in_maps, core_ids=list(range(N_CORES)))
    out = np.stack([res.results[c]["y"] for c in range(N_CORES)])
    return out.reshape(B, C, H, W).astype(np.float32)


if __name__ == "__main__":
    # quick self-check against a tiny numpy reference
    rng = np.random.default_rng(0)
    inputs = dict(
        x=rng.standard_normal((32, C, H, W), dtype=np.float32),
        w1=(rng.standard_normal((M, C, 1, 1)) * 0.05).astype(np.float32),
        g1=np.ones(M, np.float32), b1=np.zeros(M, np.float32),
        m1=(rng.standard_normal(M) * 0.1).astype(np.float32),
        v1=rng.uniform(0.5, 1.5, M).astype(np.float32),
        wd=(rng.standard_normal((M, 1, 3, 3)) * 0.1).astype(np.float32),
        g2=np.ones(M, np.float32), b2=np.zeros(M, np.float32),
        m2=(rng.standard_normal(M) * 0.1).astype(np.float32),
        v2=rng.uniform(0.5, 1.5, M).astype(np.float32),
        w3=(rng.standard_normal((C, M, 1, 1)) * 0.05).astype(np.float32),
        g3=np.ones(C, np.float32), b3=np.zeros(C, np.float32),
        m3=np.zeros(C, np.float32), v3=np.ones(C, np.float32),
    )
    out = kernel(**inputs)
    print("kernel out", out.shape, out.dtype)
